# revision 37
# baseline (speedup 1.0000x reference)
"""Trainium2 Bass kernel for nn_Attention_90658169684243.

Attention-LSTM decoder: 3x3 conv (512->512) over [B,512,8,32] feature maps,
26 sequential steps of {additive attention over 256 spatial positions,
2-layer LSTM}, and a linear head.

Sharding: data-parallel over batch across 8 cores (B=256 -> 32/core), all
parameters replicated. The wall time is dominated by the host->device
tunnel (~80MB/s), so wire bytes are minimized:
  * replicated weights are packed into one bf16 blob; each core receives
    1/8th and the kernel AllGathers the full blob over NeuronLink;
  * the feature map ships as int8 with per-channel scales folded into the
    conv weights (the conv is its only consumer);
  * remaining small tensors are packed into two arrays, and the zero
    output buffers are created on-device once and reused.
The PJRT executable is built once and memoized (run_bass_kernel_spmd would
retrace + recompile on every call).
bf16 on the matmul path with fp32 PSUM accumulation; softmax and LSTM cell
math in fp32. Sigmoid is computed as 0.5*tanh(0.5x)+0.5 so the whole kernel
uses one ACT table set (exp/tanh).
"""

import numpy as np
import ml_dtypes

bfnp = ml_dtypes.bfloat16

NCORES = 8
BFULL = 256
B = BFULL // NCORES   # 32 per core
C = 512
HF, WF = 8, 32
HW = HF * WF          # 256
T = 26
HS = 512
NCLS = 38
G4 = 4 * HS           # 2048

# Packed replicated-weight blob: (name, shape), all bf16, concatenated in
# C-order. Device offsets and host packing both derive from this table.
_BLOB_SPEC = [
    ("w9d", (3, 3, 4, 128, C)),
    ("i2hT", (4, 128, HS)),
    ("h2hT", (4, 128, HS)),
    ("w1x1T", (4, 128, HS)),
    ("hlinT", (4, 128, HS)),
    ("wih1T", (4, 128, G4)),
    ("whh1T", (4, 128, G4)),
    ("wih2T", (4, 128, G4)),
    ("whh2T", (4, 128, G4)),
    ("tail1T", (NCLS + 1, G4)),
    ("gen_wT", (4, 128, NCLS)),
    ("hlin_b", (1, HS)),
    ("h2hb", (1, HS)),
    ("b2row", (1, G4)),
    ("wsc_rep", (4, 128, B)),
    ("ident", (128, 128)),
]
_BLOB_OFF = {}
_off = 0
for _n, _s in _BLOB_SPEC:
    _sz = int(np.prod(_s))
    _BLOB_OFF[_n] = (_off, _sz)
    _off += _sz
BLOB_TOT = _off
assert BLOB_TOT % NCORES == 0, BLOB_TOT
CHUNK = BLOB_TOT // NCORES
# 12-bit packed wire format for the weight blob: per-512-group scales,
# qu in [0,4095] stored as an 8-bit lo plane + 4-bit hi nibbles packed two
# per byte (elements e and e+256 of a group share a byte). Every tensor in
# _BLOB_SPEC is a multiple of 512 elements, so groups never straddle
# tensors. 12-bit grouped is ~7x more accurate than bf16 at 75% the bytes.
NG = BLOB_TOT // 512
HIOFF = BLOB_TOT                       # byte offset of the hi-nibble plane
PBLOB = BLOB_TOT + BLOB_TOT // 2       # packed bytes total
assert PBLOB % NCORES == 0 and NG % 8 == 0
CHUNK8 = PBLOB // NCORES

# Small per-core tensors, packed into one f32 and one bf16 array.
_MISCF_SPEC = [("conv_b", 4 * 128), ("b1x1", 4 * 128), ("gen_b", NCLS)]
_MISCB_SPEC = [("bhm", 4 * 128 * B), ("h0T", 4 * 128 * B),
               ("oneh", (NCLS + 1) * T * B), ("c0", B * HS)]


def _mk_off(spec):
    d, off = {}, 0
    for n, sz in spec:
        d[n] = (off, sz)
        off += sz
    return d, off


_MISCF_OFF, MISCF_TOT = _mk_off(_MISCF_SPEC)
_MISCB_OFF, MISCB_TOT = _mk_off(_MISCB_SPEC)

_CACHE = {}


def _build():
    import contextlib

    import concourse.bacc as bacc
    import concourse.mybir as mybir
    from concourse import tile

    dt = mybir.dt
    f32 = dt.float32
    bf = dt.bfloat16
    AF = mybir.ActivationFunctionType
    OP = mybir.AluOpType

    nc = bacc.Bacc(None, num_devices=NCORES)

    def din(name, shape, dtype=bf):
        return nc.dram_tensor(name, shape, dtype, kind="ExternalInput")

    fm_ci = din("fm_ci", [4, 128, B, HF, WF], dt.int8)
    wchunk = din("wchunk", [CHUNK8], dt.uint8)
    wscales = din("wscales", [NG], f32)
    # small per-core tensors packed into two arrays (fewer PJRT args)
    miscf = din("miscf", [MISCF_TOT], f32)
    miscb = din("miscb", [MISCB_TOT])

    def mf(name, idx=0, size=None):
        off, tot = _MISCF_OFF[name]
        size = tot if size is None else size
        a = off + idx * size
        return miscf[a:a + size]

    def mb(name, idx=0, size=None):
        off, tot = _MISCB_OFF[name]
        size = tot if size is None else size
        a = off + idx * size
        return miscb[a:a + size]

    # f16 output halves the fetched bytes; |probs| <= ~1 so f16's 10-bit
    # mantissa costs < 5e-4 absolute — negligible vs the int8-fm noise.
    probsT = nc.dram_tensor("probsT", [NCLS, T * B], dt.float16,
                            kind="ExternalOutput")

    with tile.TileContext(nc) as tc:
        stack = contextlib.ExitStack()
        dram = stack.enter_context(tc.tile_pool(name="dram", bufs=1, space="DRAM"))
        const = stack.enter_context(tc.tile_pool(name="const", bufs=1))
        big = stack.enter_context(tc.tile_pool(name="big", bufs=1))
        state = stack.enter_context(tc.tile_pool(name="state", bufs=2))

        # ---- AllGather the 12-bit packed weight blob over NeuronLink ----
        bin_t = dram.tile([CHUNK8], dt.uint8, name="bin_t")
        bout = dram.tile([PBLOB], dt.uint8, addr_space="Shared", name="bout")
        wdq = dram.tile([BLOB_TOT], bf, name="wdq")
        nc.gpsimd.dma_start(bin_t[:], wchunk[:])
        nc.gpsimd.collective_compute(
            "AllGather", mybir.AluOpType.bypass,
            replica_groups=[list(range(NCORES))],
            ins=[bin_t[:].opt()], outs=[bout[:].opt()],
        )
        # dequantize 12-bit -> bf16 into DRAM scratch (one pass, ~1ms)
        with tc.tile_pool(name="dq", bufs=3) as dq:
            g0 = 0
            while g0 < NG:
                p = min(128, NG - g0)
                lo_u = dq.tile([p, 512], dt.uint8, tag="lo", name="lo_u")
                hp_u = dq.tile([p, 256], dt.uint8, tag="hp", name="hp_u")
                scl = dq.tile([p, 1], f32, tag="scl", name="scl")
                nc.sync.dma_start(lo_u[:], bout[g0 * 512:(g0 + p) * 512])
                nc.sync.dma_start(hp_u[:],
                                  bout[HIOFF + g0 * 256:HIOFF + (g0 + p) * 256])
                nc.sync.dma_start(scl[:], wscales[g0:g0 + p])
                h0u = dq.tile([p, 256], dt.uint8, tag="h0", name="h0u")
                h1u = dq.tile([p, 256], dt.uint8, tag="h1", name="h1u")
                nc.vector.tensor_scalar(h0u[:], hp_u[:], 15, None,
                                        OP.bitwise_and)
                nc.vector.tensor_scalar(h1u[:], hp_u[:], 4, None,
                                        OP.logical_shift_right)
                lof = dq.tile([p, 512], f32, tag="lof", name="lof")
                h0f = dq.tile([p, 256], f32, tag="h0f", name="h0f")
                h1f = dq.tile([p, 256], f32, tag="h1f", name="h1f")
                nc.vector.tensor_copy(lof[:], lo_u[:])
                nc.vector.tensor_copy(h0f[:], h0u[:])
                nc.vector.tensor_copy(h1f[:], h1u[:])
                qu = dq.tile([p, 512], f32, tag="qu", name="qu")
                nc.vector.tensor_scalar(qu[:, 0:256], h0f[:], 256.0, None,
                                        OP.mult)
                nc.vector.tensor_scalar(qu[:, 256:512], h1f[:], 256.0, None,
                                        OP.mult)
                nc.vector.tensor_tensor(qu[:, 0:256], qu[:, 0:256],
                                        lof[:, 0:256], OP.add)
                nc.vector.tensor_tensor(qu[:, 256:512], qu[:, 256:512],
                                        lof[:, 256:512], OP.add)
                wbf = dq.tile([p, 512], bf, tag="wbf", name="wbf")
                nc.vector.tensor_scalar(wbf[:], qu[:], -2048.0, scl[:, 0:1],
                                        OP.add, OP.mult)
                nc.sync.dma_start(wdq[g0 * 512:(g0 + p) * 512], wbf[:])
                g0 += p

        def bl(name, idx=0, size=None):
            """AP into the dequantized blob for tensor `name`, element
            offset idx*size within it (size defaults to the whole tensor)."""
            off, tot = _BLOB_OFF[name]
            if size is None:
                size = tot
            a = off + idx * size
            return wdq[a:a + size]

        fmh = [big.tile([128, B, HW], bf, tag=f"fmh{i}", name=f"fmh{i}")
               for i in range(4)]
        fmhT = [big.tile([128, B, C], bf, tag=f"fmhT{i}", name=f"fmhT{i}")
                for i in range(2)]

        def cload(name, src, shape, dtype=bf, pool=None):
            t = (pool or const).tile(shape, dtype, tag=name, name=name)
            nc.sync.dma_start(t[:], src)
            return t

        ones = const.tile([1, B], bf, tag="ones", name="ones")
        nc.vector.memset(ones[:], 1.0)
        ones128 = const.tile([128, B], bf, tag="ones128", name="ones128")
        nc.vector.memset(ones128[:], 1.0)
        bh_plus = const.tile([B, HS], f32, tag="bh_plus", name="bh_plus")

        # ---------------- phase 1: conv (+ bh_proj) ----------------
        with (
            tc.tile_pool(name="cpad", bufs=1) as cpad,
            tc.tile_pool(name="cw", bufs=1) as cw,
            tc.tile_pool(name="cps", bufs=4, space="PSUM") as cps,
            tc.tile_pool(name="cpt", bufs=4, space="PSUM") as cpt,
        ):
            ident = cw.tile([128, 128], bf, tag="ident", name="ident")
            nc.sync.dma_start(ident[:], bl("ident"))
            conv_b = []
            for k in range(4):
                cb = cw.tile([128, 1], f32, tag=f"conv_b{k}", name=f"conv_b{k}")
                nc.sync.dma_start(cb[:], mf("conv_b", k, 128))
                conv_b.append(cb)
            w9 = [[[cw.tile([128, C], bf, tag=f"w9_{kh}{kw}{ci}",
                            name=f"w9_{kh}{kw}{ci}")
                    for ci in range(4)] for kw in range(3)] for kh in range(3)]
            for kh in range(3):
                for kw in range(3):
                    for ci in range(4):
                        nc.gpsimd.dma_start(
                            w9[kh][kw][ci][:],
                            bl("w9d", (kh * 3 + kw) * 4 + ci, 128 * C))

            BC = 2  # batch chunk for conv
            for bc in range(B // BC):
                b0 = bc * BC
                pads = []
                for ci in range(4):
                    pad = cpad.tile([128, BC, HF + 2, WF + 2], bf,
                                    tag=f"pad{ci}", name=f"pad{ci}")
                    nc.vector.memset(pad[:, :, 0, :], 0.0)
                    nc.vector.memset(pad[:, :, HF + 1, :], 0.0)
                    nc.vector.memset(pad[:, :, 1:HF + 1, 0], 0.0)
                    nc.vector.memset(pad[:, :, 1:HF + 1, WF + 1], 0.0)
                    for b in range(BC):
                        nc.gpsimd.dma_start(pad[:, b, 1:HF + 1, 1:WF + 1],
                                            fm_ci[ci, :, b0 + b])
                    pads.append(pad)
                for co in range(4):
                    ps = cps.tile([128, BC, HW], f32, tag="pscv", name="pscv")
                    idx = 0
                    for kh in range(3):
                        for kw in range(3):
                            for ci in range(4):
                                nc.tensor.matmul(
                                    ps[:],
                                    w9[kh][kw][ci][:, co * 128:(co + 1) * 128],
                                    pads[ci][:, :, kh:kh + HF, kw:kw + WF],
                                    start=(idx == 0), stop=(idx == 35))
                                idx += 1
                    for b in range(BC):
                        nc.vector.tensor_scalar_add(
                            fmh[co][:, b0 + b, :], ps[:, b, :],
                            conv_b[co][:, 0:1])
                    for b in range(BC):
                        for hh in range(2):
                            pt = cpt.tile([128, 128], bf, tag="pst", name="pst")
                            nc.tensor.transpose(
                                pt[:],
                                fmh[co][:, b0 + b, hh * 128:(hh + 1) * 128],
                                ident[:])
                            nc.vector.tensor_copy(
                                fmhT[hh][:, b0 + b, co * 128:(co + 1) * 128],
                                pt[:])

        # ---- bh_proj_plus = mean_t(batch_H) @ i2h^T + h2h_b (once) ----
        with (
            tc.tile_pool(name="pre", bufs=1) as pre,
            tc.tile_pool(name="prep", bufs=1, space="PSUM") as prep,
        ):
            i2h = [pre.tile([128, HS], bf, tag=f"i2h{k}", name=f"i2h{k}")
                   for k in range(4)]
            bhm = [pre.tile([128, B], bf, tag=f"bhm{k}", name=f"bhm{k}")
                   for k in range(4)]
            h2hb_row = pre.tile([1, HS], bf, tag="h2hb_row", name="h2hb_row")
            nc.sync.dma_start(h2hb_row[:], bl("h2hb"))
            for k in range(4):
                nc.gpsimd.dma_start(i2h[k][:], bl("i2hT", k, 128 * HS))
                nc.gpsimd.dma_start(bhm[k][:], mb("bhm", k, 128 * B))
            ps_bh = prep.tile([B, HS], f32, tag="psbh", name="psbh")
            for k in range(4):
                nc.tensor.matmul(ps_bh[:], bhm[k][:], i2h[k][:],
                                 start=(k == 0), stop=False)
            nc.tensor.matmul(ps_bh[:], ones[:], h2hb_row[:],
                             start=False, stop=True)
            nc.vector.tensor_copy(bh_plus[:], ps_bh[:])

        # ---------------- phase 2: 26-step scan ----------------
        wconst = stack.enter_context(tc.tile_pool(name="wconst", bufs=1))
        h2hT = [cload(f"h2hT{k}", bl("h2hT", k, 128 * HS), [128, HS],
                      pool=wconst) for k in range(4)]
        w1x1T = [cload(f"w1x1T{k}", bl("w1x1T", k, 128 * HS), [128, HS],
                       pool=wconst) for k in range(4)]
        b1x1T = [cload(f"b1x1T{k}", mf("b1x1", k, 128), [128, 1], f32,
                       pool=wconst) for k in range(4)]
        hlinT = [cload(f"hlinT{k}", bl("hlinT", k, 128 * HS), [128, HS],
                       pool=wconst) for k in range(4)]
        h1T = [cload(f"h1T_{k}", mb("h0T", k, 128 * B), [128, B], pool=wconst)
               for k in range(4)]
        h2T = [cload(f"h2T_{k}", mb("h0T", k, 128 * B), [128, B], pool=wconst)
               for k in range(4)]
        c1 = wconst.tile([B, HS], f32, tag="c1", name="c1")
        c2 = wconst.tile([B, HS], f32, tag="c2", name="c2")
        nc.gpsimd.dma_start(c1[:], mb("c0"))  # bf16 -> f32 cast DMA
        nc.gpsimd.dma_start(c2[:], mb("c0"))
        hlin_b = cload("hlin_b", bl("hlin_b"), [1, HS], pool=wconst)
        tail1T = cload("tail1T", bl("tail1T"), [NCLS + 1, G4], pool=wconst)
        b2r = cload("b2r", bl("b2row"), [1, G4], pool=wconst)
        wsc_rep = [cload(f"wsc_rep{k}", bl("wsc_rep", k, 128 * B), [128, B],
                         pool=wconst) for k in range(4)]
        gen_wT = [cload(f"gen_wT{k}", bl("gen_wT", k, 128 * NCLS), [128, NCLS],
                        pool=wconst) for k in range(4)]
        gen_bT = cload("gen_bT", mf("gen_b"), [NCLS, 1], f32, pool=wconst)
        oneh = cload("oneh", mb("oneh"), [NCLS + 1, T, B], pool=wconst)
        h2all = [big.tile([128, T * B], bf, tag=f"h2all{i}", name=f"h2all{i}")
                 for i in range(4)]
        sb = stack.enter_context(tc.tile_pool(name="sb", bufs=2))
        sb1 = stack.enter_context(tc.tile_pool(name="sb1", bufs=1))
        tp = stack.enter_context(tc.tile_pool(name="tp", bufs=2))
        ws = stack.enter_context(tc.tile_pool(name="ws", bufs=2))
        mm = stack.enter_context(tc.tile_pool(name="mm", bufs=2, space="PSUM"))

        for t in range(T):
            # ---- v = h2 @ h2h_w^T + (bh_proj + h2h_b) ----
            ps_v = mm.tile([B, HS], f32, tag="mm", name="mm")
            for k in range(4):
                nc.tensor.matmul(ps_v[:], h2T[k][:, :], h2hT[k][:],
                                 start=(k == 0), stop=(k == 3))
            v_bf = sb1.tile([B, HS], bf, tag="vb", name="v_bf")
            nc.vector.tensor_tensor(v_bf[:], ps_v[:], bh_plus[:], OP.add)
            vT = [sb.tile([128, B], bf, tag=f"vT{k}", name=f"vT{k}")
                  for k in range(4)]
            t32(nc, vT, v_bf[:], HS)

            # ---- q = v @ w1x1^T (bias folded into attention tanh) ----
            ps_q = mm.tile([B, HS], f32, tag="mm", name="mm")
            for k in range(4):
                nc.tensor.matmul(ps_q[:], vT[k][:], w1x1T[k][:],
                                 start=(k == 0), stop=(k == 3))
            q_sb = sb1.tile([B, HS], f32, tag="th4", name="q_sb")
            nc.vector.tensor_copy(q_sb[:], ps_q[:])
            qT = [sb.tile([128, B], f32, tag=f"qT{k}", name=f"qT{k}")
                  for k in range(4)]
            t32(nc, qT, q_sb[:], HS)
            # qb1[ct][:, b] = q[b, ct-chunk] + b1x1[ct-chunk]: per-partition
            # bias for the attention tanh (fused into the ACT op below).
            qb1 = [sb.tile([128, B], f32, tag=f"qb1_{k}", name=f"qb1_{k}")
                   for k in range(4)]
            for k in range(4):
                nc.vector.tensor_scalar_add(qb1[k][:], qT[k][:],
                                            b1x1T[k][:, 0:1])

            # ---- e[b, hw] = sum_c wsc_c * tanh(fmh + q + b1x1) ----
            # lhsT = w_score replicated over 32 cols -> all PSUM rows
            # identical; row bb at free block i is e for batch bb, so the
            # extraction copy stays on one partition.
            e_sb = sb1.tile([B, HW], f32, tag="e_sb", name="e_sb")
            for g in range(8):        # groups of 4 batch rows
                gb = g * 4
                ps_e = mm.tile([B, 4, HW], f32, tag="mm", name="mm")
                for ct in range(4):
                    for nb in range(2):
                        tt = tp.tile([128, 2, HW], bf, tag="t", name="t")
                        for i2 in range(2):
                            i = nb * 2 + i2
                            nc.scalar.activation(
                                tt[:, i2, :], fmh[ct][:, gb + i, :], AF.Tanh,
                                bias=qb1[ct][:, gb + i:gb + i + 1])
                        nc.tensor.matmul(
                            ps_e[:, nb * 2:nb * 2 + 2, :],
                            wsc_rep[ct][:],
                            tt[:],
                            start=(ct == 0), stop=(ct == 3))
                # all PSUM rows identical: stage row 0 to SBUF, then DMA
                # scatters the four b-rows to their partitions.
                # HW quirk: ACT copies with multi-dim free APs from PSUM
                # corrupt the 2nd block, and 1->N-partition scatter DMAs with
                # multi-dim source APs misplace data -> do both per row.
                for half in range(2):
                    es = sb.tile([1, 2, HW], f32, tag="es", name="es")
                    for i2 in range(2):
                        r = half * 2 + i2
                        nc.scalar.copy(es[:, i2, :], ps_e[0:1, r, :])
                        nc.scalar.dma_start(e_sb[gb + r:gb + r + 1, :],
                                            es[0:1, i2, :])

            # ---- softmax over hw (score_b dropped: shift-invariant) ----
            neg_m = sb.tile([B, 1], f32, tag="neg_m", name="neg_m")
            nc.vector.tensor_reduce(neg_m[:], e_sb[:], mybir.AxisListType.X,
                                    OP.max, negate=True)
            expz = sb.tile([B, HW], f32, tag="es", name="expz")
            nc.scalar.activation(expz[:], e_sb[:], AF.Exp, bias=neg_m[:, 0:1])
            zsum = sb.tile([B, 1], f32, tag="zsum", name="zsum")
            nc.vector.tensor_reduce(zsum[:], expz[:], mybir.AxisListType.X,
                                    OP.add)
            rz = sb.tile([B, 1], f32, tag="rz", name="rz")
            nc.vector.reciprocal(rz[:], zsum[:])
            alpha = sb1.tile([B, HW], f32, tag="e_sb", name="alpha")
            nc.vector.tensor_scalar_mul(alpha[:], expz[:], rz[:, 0:1])
            alphaT = [sb.tile([128, B], f32, tag=f"alphaT{k}", name=f"alphaT{k}")
                      for k in range(2)]
            t32(nc, alphaT, alpha[:], HW)

            # ---- context[b, c] = sum_hw alpha * fmh ----
            # lhsT = full alphaT [128, 32]: PSUM row b' uses alpha_b'; the
            # diagonal row b' = bb at free block i is the true context.
            ctx_bf = sb1.tile([B, HS], bf, tag="vb", name="ctx_bf")
            for g in range(8):        # groups of 4 batch rows
                ps_c = mm.tile([B, 4, HS], f32, tag="mm", name="mm")
                for i in range(4):
                    bb = g * 4 + i
                    for kt in range(2):
                        # replicate alphaT column bb across 32 lhsT columns
                        # so every PSUM row holds context for batch bb
                        arep = sb.tile([128, B], bf, tag=f"arep{kt}",
                                       name=f"arep{kt}")
                        nc.vector.tensor_scalar(
                            arep[:], ones128[:],
                            alphaT[kt][:, bb:bb + 1], None, OP.mult)
                        nc.tensor.matmul(
                            ps_c[:, i, :],
                            arep[:],
                            fmhT[kt][:, bb, :],
                            start=(kt == 0), stop=(kt == 1))
                for half in range(2):
                    cs = sb.tile([1, 2, HS], bf, tag="cs", name="cs")
                    for i2 in range(2):
                        r = half * 2 + i2
                        nc.scalar.copy(cs[:, i2, :], ps_c[0:1, r, :])
                        nc.scalar.dma_start(
                            ctx_bf[g * 4 + r:g * 4 + r + 1, :],
                            cs[0:1, i2, :])
            xT = [sb.tile([128, B], bf, tag=f"xT{k}", name=f"xT{k}")
                  for k in range(4)]
            t32(nc, xT, ctx_bf[:], HS)

            # ---- LSTM 1 gates (k-outer so streamed weights die fast) ----
            ps_g = mm.tile([B, G4], f32, tag="mm", name="mm")
            for k in range(4):
                w = ws.tile([128, G4], bf, tag="ws", name="ws")
                nc.gpsimd.dma_start(w[:], bl("wih1T", k, 128 * G4))
                for nb in range(4):
                    nc.tensor.matmul(ps_g[:, nb * HS:(nb + 1) * HS], xT[k][:],
                                     w[:, nb * HS:(nb + 1) * HS],
                                     start=(k == 0), stop=False)
            for nb in range(4):
                nc.tensor.matmul(ps_g[:, nb * HS:(nb + 1) * HS],
                                 oneh[:, t, :], tail1T[:, nb * HS:(nb + 1) * HS],
                                 start=False, stop=False)
            for k in range(4):
                w = ws.tile([128, G4], bf, tag="ws", name="ws")
                nc.gpsimd.dma_start(w[:], bl("whh1T", k, 128 * G4))
                for nb in range(4):
                    nc.tensor.matmul(ps_g[:, nb * HS:(nb + 1) * HS], h1T[k][:],
                                     w[:, nb * HS:(nb + 1) * HS],
                                     start=False, stop=(k == 3))

            def lstm_cell(ps, c_prev, tag):
                # th4 slices: 0=i, 1=f, 2=g, 3=o
                th4 = sb1.tile([B, 4, HS], f32, tag="th4", name="th4")
                nc.scalar.activation(th4[:, 0, :], ps[:, 0:HS], AF.Tanh, scale=0.5)
                nc.scalar.activation(th4[:, 1, :], ps[:, HS:2 * HS], AF.Tanh,
                                     scale=0.5)
                nc.scalar.activation(th4[:, 2, :], ps[:, 2 * HS:3 * HS], AF.Tanh)
                nc.scalar.activation(th4[:, 3, :], ps[:, 3 * HS:4 * HS], AF.Tanh,
                                     scale=0.5)
                for sl in (0, 1, 3):  # sigmoid = 0.5*tanh(0.5x) + 0.5
                    nc.vector.tensor_scalar(th4[:, sl, :], th4[:, sl, :],
                                            0.5, 0.5, OP.mult, OP.add)
                nc.vector.tensor_tensor(th4[:, 1, :], th4[:, 1, :], c_prev[:],
                                        OP.mult)
                nc.vector.tensor_tensor(th4[:, 0, :], th4[:, 0, :], th4[:, 2, :],
                                        OP.mult)
                c_new = state.tile([B, HS], f32, tag=f"c{tag}", name=f"c{tag}")
                nc.vector.tensor_tensor(c_new[:], th4[:, 1, :], th4[:, 0, :],
                                        OP.add)
                nc.scalar.activation(th4[:, 2, :], c_new[:], AF.Tanh)
                h_bf = sb.tile([B, HS], bf, tag="hbf", name=f"hbf{tag}")
                nc.vector.tensor_tensor(h_bf[:], th4[:, 3, :], th4[:, 2, :],
                                        OP.mult)
                return c_new, h_bf

            c1, h1_bf = lstm_cell(ps_g, c1, "1")
            h1T = [state.tile([128, B], bf, tag=f"h1T{k}", name=f"h1T{k}")
                   for k in range(4)]
            t32(nc, h1T, h1_bf[:], HS)

            # ---- cur = h1 @ hlin_w^T + hlin_b ----
            ps_h = mm.tile([B, HS], f32, tag="mm", name="mm")
            for k in range(4):
                nc.tensor.matmul(ps_h[:], h1T[k][:], hlinT[k][:],
                                 start=(k == 0), stop=False)
            nc.tensor.matmul(ps_h[:], ones[:], hlin_b[:], start=False, stop=True)
            cur_bf = sb1.tile([B, HS], bf, tag="vb", name="cur_bf")
            nc.scalar.copy(cur_bf[:], ps_h[:])
            curT = [sb.tile([128, B], bf, tag=f"curT{k}", name=f"curT{k}")
                    for k in range(4)]
            t32(nc, curT, cur_bf[:], HS)

            # ---- LSTM 2 gates ----
            ps_g2 = mm.tile([B, G4], f32, tag="mm", name="mm")
            for k in range(4):
                w = ws.tile([128, G4], bf, tag="ws", name="ws")
                nc.gpsimd.dma_start(w[:], bl("wih2T", k, 128 * G4))
                for nb in range(4):
                    nc.tensor.matmul(ps_g2[:, nb * HS:(nb + 1) * HS], curT[k][:],
                                     w[:, nb * HS:(nb + 1) * HS],
                                     start=(k == 0), stop=False)
            for k in range(4):
                w = ws.tile([128, G4], bf, tag="ws", name="ws")
                nc.gpsimd.dma_start(w[:], bl("whh2T", k, 128 * G4))
                for nb in range(4):
                    nc.tensor.matmul(ps_g2[:, nb * HS:(nb + 1) * HS], h2T[k][:],
                                     w[:, nb * HS:(nb + 1) * HS],
                                     start=False, stop=False)
            for nb in range(4):
                nc.tensor.matmul(ps_g2[:, nb * HS:(nb + 1) * HS], ones[:],
                                 b2r[:, nb * HS:(nb + 1) * HS],
                                 start=False, stop=True)

            c2, h2_bf = lstm_cell(ps_g2, c2, "2")
            h2T = [h2all[k][:, t * B:(t + 1) * B] for k in range(4)]
            t32(nc, h2T, h2_bf[:], HS)

        # ---------------- phase 3: probs = h2_all @ gen_w^T + gen_b ----------------
        out_sb = sb1.tile([NCLS, T * B], f32, tag="th4", name="out_sb")
        for n0, n1 in ((0, 512), (512, T * B)):
            ps_p = mm.tile([NCLS, n1 - n0], f32, tag="mm", name="mm")
            for k in range(4):
                nc.tensor.matmul(ps_p[:], gen_wT[k][:], h2all[k][:, n0:n1],
                                 start=(k == 0), stop=(k == 3))
            nc.scalar.activation(out_sb[:, n0:n1], ps_p[:], AF.Identity,
                                 bias=gen_bT[:, 0:1])
        nc.gpsimd.dma_start(probsT[:], out_sb[:])  # f32 -> f16 cast DMA

        stack.close()

    nc.compile()
    return nc


def t32(nc, dst_tiles, src_ap, ncols):
    """Transpose src [32, ncols] into tiles of [128, 32] via DVE 32x32 block
    transposes: block j of src lands at dst_tiles[j // 4] rows (j % 4)*32."""
    for j in range(ncols // 32):
        kt, r = j // 4, (j % 4) * 32
        nc.vector.transpose(dst_tiles[kt][r:r + 32, :],
                            src_ap[:, j * 32:(j + 1) * 32])


def _quant_fm(inputs):
    """Per-channel symmetric int8 quantization of feature_map. The scales
    are folded into the conv weights host-side (the conv is the only
    consumer of the raw feature map), so the device just casts int8->bf16."""
    f32 = np.float32
    fm = np.asarray(inputs["feature_map"], f32)
    s = np.abs(fm).max(axis=(0, 2, 3)) / 127.0   # [C]
    s = np.maximum(s, 1e-30)
    buf = fm * (1.0 / s)[None, :, None, None]
    np.rint(buf, out=buf)
    np.clip(buf, -127, 127, out=buf)
    fmq = buf.astype(np.int8)
    return fmq, s


def _pack_blob(inputs, fm_scale):
    """Host-side: pack all replicated weights into one bf16 blob matching
    _BLOB_SPEC order (device reads slices of the AllGathered copy)."""
    f32 = np.float32
    w9d = np.asarray(inputs["conv_m2h_w"], f32).transpose(2, 3, 1, 0).reshape(
        3, 3, 4, 128, C)
    # fold the fm int8 dequant scales into the conv weights (contract dim
    # is c_in = axes 2-3 of w9d)
    w9d = w9d * fm_scale.reshape(4, 128)[None, None, :, :, None]
    b1 = np.asarray(inputs["rnn1_b_ih"], f32) + np.asarray(inputs["rnn1_b_hh"], f32)
    b2 = np.asarray(inputs["rnn2_b_ih"], f32) + np.asarray(inputs["rnn2_b_hh"], f32)
    wih1T = np.asarray(inputs["rnn1_w_ih"], f32).T
    tail1T = np.concatenate([wih1T[512:550], b1[None]], axis=0)
    wsc = np.asarray(inputs["score_w"], f32)[0, :, 0, 0]
    parts = {
        "w9d": w9d,
        "i2hT": np.asarray(inputs["i2h_w"], f32).T.reshape(4, 128, HS),
        "h2hT": np.asarray(inputs["h2h_w"], f32).T.reshape(4, 128, HS),
        "w1x1T": np.asarray(inputs["conv_h2h_w"], f32)[:, :, 0, 0].T
                 .reshape(4, 128, HS),
        "hlinT": np.asarray(inputs["hlin_w"], f32).T.reshape(4, 128, HS),
        "wih1T": wih1T[:512].reshape(4, 128, G4),
        "whh1T": np.asarray(inputs["rnn1_w_hh"], f32).T.reshape(4, 128, G4),
        "wih2T": np.asarray(inputs["rnn2_w_ih"], f32).T.reshape(4, 128, G4),
        "whh2T": np.asarray(inputs["rnn2_w_hh"], f32).T.reshape(4, 128, G4),
        "tail1T": tail1T,
        "gen_wT": np.asarray(inputs["gen_w"], f32).T.reshape(4, 128, NCLS),
        "hlin_b": np.asarray(inputs["hlin_b"], f32)[None],
        "h2hb": np.asarray(inputs["h2h_b"], f32)[None],
        "b2row": b2[None],
        "wsc_rep": np.tile(wsc.reshape(4, 128, 1), (1, 1, B)),
        "ident": np.eye(128, dtype=f32),
    }
    blob = np.empty(BLOB_TOT, f32)
    for name, shape in _BLOB_SPEC:
        off, sz = _BLOB_OFF[name]
        arr = np.ascontiguousarray(parts[name], dtype=f32).reshape(-1)
        assert arr.size == sz, (name, arr.size, sz)
        blob[off:off + sz] = arr
    # 12-bit encode: per-512-group absmax scales, lo byte plane + packed
    # hi nibbles (elements e and e+256 of a group share a byte)
    g = blob.reshape(NG, 512)
    s = np.maximum(np.abs(g).max(axis=1, keepdims=True) / 2047.0, 1e-30)
    q = np.clip(np.rint(g / s), -2047, 2047).astype(np.int32) + 2048
    lo = (q & 255).astype(np.uint8)
    hp = ((q >> 8)[:, :256] | ((q >> 8)[:, 256:] << 4)).astype(np.uint8)
    packed = np.concatenate([lo.reshape(-1), hp.reshape(-1)])
    return packed, s.reshape(-1).astype(f32)


def _prep_all(inputs):
    """Build the global (concatenated-over-cores) input arrays directly."""
    f32 = np.float32
    fmq, fm_scale = _quant_fm(inputs)
    packed, wsc = _pack_blob(inputs, fm_scale)
    fm_g = np.ascontiguousarray(
        fmq.reshape(NCORES, B, 4, 128, HF, WF).transpose(0, 2, 3, 1, 4, 5)
    ).reshape(NCORES * 4, 128, B, HF, WF)

    bhm = np.asarray(inputs["batch_H"], f32).mean(axis=1)      # [256, 512]
    hh = np.asarray(inputs["hidden_h"], f32)
    hc = np.asarray(inputs["hidden_c"], f32)
    h0 = (hh[0] + hh[1]) * 0.5
    c0 = (hc[0] + hc[1]) * 0.5
    text = np.asarray(inputs["text"])

    miscf_g = np.empty((NCORES, MISCF_TOT), f32)
    miscb_g = np.empty((NCORES, MISCB_TOT), bfnp)
    for c in range(NCORES):
        sl = slice(c * B, (c + 1) * B)
        onehT = np.zeros((NCLS + 1, T, B), f32)
        for b in range(B):
            for t in range(T):
                onehT[int(text[c * B + b, t]), t, b] = 1.0
        onehT[NCLS] = 1.0
        for name, arr in (("conv_b", inputs["conv_m2h_b"]),
                          ("b1x1", inputs["conv_h2h_b"]),
                          ("gen_b", inputs["gen_b"])):
            off, sz = _MISCF_OFF[name]
            miscf_g[c, off:off + sz] = np.asarray(arr, f32).reshape(-1)
        for name, arr in (("bhm", bhm[sl].T.reshape(4, 128, B)),
                          ("h0T", h0[sl].T.reshape(4, 128, B)),
                          ("oneh", onehT), ("c0", c0[sl])):
            off, sz = _MISCB_OFF[name]
            miscb_g[c, off:off + sz] = np.ascontiguousarray(arr, f32).reshape(-1)

    return {
        "fm_ci": fm_g,
        "wchunk": packed,
        "wscales": np.tile(wsc, NCORES),
        "miscf": miscf_g.reshape(-1),
        "miscb": miscb_g.reshape(-1),
    }


def _get_runner():
    """Persistent PJRT executable for the Bass module (compile once).

    run_bass_kernel_spmd under axon rebuilds a fresh jax.jit closure per
    call — every invocation pays retrace + NeuronCC compile. This builds
    the same shard_map'd _bass_exec executable once and memoizes it, so
    steady-state executions only pay transfer + dispatch + HW exec.
    """
    if "runner" in _CACHE:
        return _CACHE["runner"]

    import jax
    import jax.numpy as jnp
    import concourse.mybir as mybir
    from concourse import bass2jax
    from jax.experimental.shard_map import shard_map
    from jax.sharding import Mesh, NamedSharding, PartitionSpec

    nc = _CACHE["nc"]
    bass2jax.install_neuronx_cc_hook()

    partition_name = (nc.partition_id_tensor.name
                      if nc.partition_id_tensor else None)
    in_names, out_names, out_avals, zero_shapes = [], [], [], []
    for alloc in nc.m.functions[0].allocations:
        if not isinstance(alloc, mybir.MemoryLocationSet):
            continue
        name = alloc.memorylocations[0].name
        if alloc.kind == "ExternalInput":
            if name != partition_name:
                in_names.append(name)
        elif alloc.kind == "ExternalOutput":
            out_names.append(name)
            shape = tuple(alloc.tensor_shape)
            dtype = mybir.dt.np(alloc.dtype)
            out_avals.append(jax.core.ShapedArray(shape, dtype))
            zero_shapes.append((shape, dtype))
    n_params = len(in_names)
    all_names = list(in_names) + list(out_names)
    if partition_name is not None:
        all_names.append(partition_name)

    def _body(*args):
        operands = list(args)
        if partition_name is not None:
            operands.append(bass2jax.partition_id_tensor())
        outs = bass2jax._bass_exec_p.bind(
            *operands,
            out_avals=tuple(out_avals),
            in_names=tuple(all_names),
            out_names=tuple(out_names),
            lowering_input_output_aliases=(),
            sim_require_finite=True,
            sim_require_nnan=True,
            nc=nc,
        )
        return tuple(outs)

    devices = jax.devices()[:NCORES]
    mesh = Mesh(np.asarray(devices), ("core",))
    n_outs = len(out_names)
    sharded = jax.jit(
        shard_map(_body, mesh=mesh,
                  in_specs=(PartitionSpec("core"),) * (n_params + n_outs),
                  out_specs=(PartitionSpec("core"),) * n_outs,
                  check_rep=False),
        keep_unused=True,
    )
    # The output buffers must be zero-filled jit parameters (the neuronx_cc
    # hook requires bass_exec operands to be literal parameters), but their
    # CONTENT is produced on-device once and reused: the kernel writes every
    # element of its outputs, so the zero buffers are never consumed
    # (not donated) and need no per-call upload or dispatch.
    zsh = NamedSharding(mesh, PartitionSpec("core"))
    zjit = jax.jit(
        lambda: tuple(jnp.zeros((NCORES * s[0], *s[1:]), d)
                      for s, d in zero_shapes),
        out_shardings=tuple(zsh for _ in zero_shapes),
    )
    zeros_dev = zjit()
    jax.block_until_ready(zeros_dev)

    def run(gl):
        out_arrs = sharded(*[gl[name] for name in in_names], *zeros_dev)
        return {name: np.asarray(out_arrs[i]).reshape(
                    NCORES, *zero_shapes[i][0])
                for i, name in enumerate(out_names)}

    run.sharded = sharded
    run.in_names = in_names
    run.out_names = out_names
    run.zero_shapes = zero_shapes
    run.mesh = mesh
    _CACHE["runner"] = run
    return run


def kernel(**inputs):
    if "nc" not in _CACHE:
        _CACHE["nc"] = _build()

    gl = _prep_all(inputs)
    results = _get_runner()(gl)
    # probsT: [NCORES, NCLS, T*B] f16 -> [BFULL, T, NCLS] f32
    out = np.empty((BFULL, T, NCLS), np.float32)
    pr = results["probsT"].astype(np.float32).reshape(NCORES, NCLS, T, B)
    for c in range(NCORES):
        out[c * B:(c + 1) * B] = pr[c].transpose(2, 1, 0)
    return out


if __name__ == "__main__":
    _build()
    print("build ok")
